# revision 1
# baseline (speedup 1.0000x reference)
"""GQA attention kernel for Trainium2, sharded over 8 NeuronCores.

Sharding: core c = b*4 + g handles batch b and GQA group g (4 query heads
+ 1 KV head). Wq/Wk/Wv column-sharded per group, Wo row-sharded; the host
sums the 4 per-group partial outputs per batch.

Device layout tricks:
  - x is passed transposed (xT [D, S]) so Q^T/K^T project directly into
    [head_dim, S] layout (head_dim on partitions) and V projects into
    natural [S, head_dim] layout.
  - Q/K head dims are de-interleaved host-side (even dims then odd dims)
    by permuting Wq/Wk columns, making RoPE a half-tile multiply/add.
    Scores are invariant to a shared permutation of Q/K dims.
  - Attention computes scoresT [key, query] so softmax exp output is
    directly the lhs^T operand ("P^T") for the P@V matmul: zero PE
    transposes anywhere.
  - 1/sqrt(dh) is folded into the exp activation's scale; the causal mask
    is a 0/1 multiply on the 4 diagonal-straddling blocks per q-block.
  - softmax denominator: DVE accumulates sum over key-chunks, PE ones-
    matmul reduces over partitions, then a K=1 ones-matmul broadcasts the
    reciprocal across partitions for the output normalization multiply.
"""

import sys

if "/opt/trn_rl_repo" not in sys.path:
    sys.path.insert(0, "/opt/trn_rl_repo")

import numpy as np
import ml_dtypes

import concourse.bass as bass
import concourse.bacc as bacc
import concourse.tile as tile
from concourse import mybir
from concourse.bass_utils import run_bass_kernel_spmd

B = 2
S = 2048
D = 2048
N_HEADS = 16
N_KV = 4
DH = 128
NH = 4  # query heads per core
N_CORES = 8

INV_SQRT_DH = 1.0 / np.sqrt(DH)
F32 = mybir.dt.float32
BF16 = mybir.dt.bfloat16


def build_program(s=S, d=D):
    """Per-core program: 4 query heads + 1 KV head of causal GQA."""
    kc_n = d // 128       # contraction chunks
    sc = 512              # projection s-chunk
    nsc = s // sc
    qb_n = s // 512       # attention q-blocks
    st_n = s // 128       # output s-tiles

    nc = bacc.Bacc("TRN2", target_bir_lowering=False, debug=False,
                   num_devices=N_CORES)
    xT = nc.declare_dram_parameter("xT", [d, s], BF16, isOutput=False)
    wq = nc.declare_dram_parameter("wq", [d, NH * DH], BF16, isOutput=False)
    wkv = nc.declare_dram_parameter("wkv", [d, 2 * DH], BF16, isOutput=False)
    wo = nc.declare_dram_parameter("wo", [NH * DH, d], F32, isOutput=False)
    cosT = nc.declare_dram_parameter("cosT", [64, s], F32, isOutput=False)
    sinT = nc.declare_dram_parameter("sinT", [64, s], F32, isOutput=False)
    maskb = nc.declare_dram_parameter("maskb", [128, 896], BF16, isOutput=False)
    out_p = nc.declare_dram_parameter("out_p", [s, d], F32, isOutput=True)

    with tile.TileContext(nc) as tc:
        with (
            tc.tile_pool(name="const", bufs=1) as cpool,
            tc.tile_pool(name="xp", bufs=1) as xpool,
            tc.tile_pool(name="act", bufs=1) as apool,
            tc.tile_pool(name="tmp", bufs=1) as tpool,
            tc.tile_pool(name="psum", bufs=1, space="PSUM") as pp,
        ):
            # ---- constants ----
            wq_sb = cpool.tile([128, kc_n, NH * DH], BF16, tag="wq")
            nc.sync.dma_start(wq_sb[:], wq.rearrange("(n p) m -> p n m", p=128))
            wkv_sb = cpool.tile([128, kc_n, 2 * DH], BF16, tag="wkv")
            nc.sync.dma_start(wkv_sb[:], wkv.rearrange("(n p) m -> p n m", p=128))
            wo_sb = cpool.tile([128, NH, d], F32, tag="wo")
            nc.sync.dma_start(wo_sb[:], wo.rearrange("(n p) m -> p n m", p=128))
            cos_sb = cpool.tile([64, s], F32, tag="cos")
            nc.sync.dma_start(cos_sb[:], cosT[:])
            sin_sb = cpool.tile([64, s], F32, tag="sin")
            nc.sync.dma_start(sin_sb[:], sinT[:])
            mask_sb = cpool.tile([128, 896], BF16, tag="mask")
            nc.sync.dma_start(mask_sb[:], maskb[:])
            ones_col = cpool.tile([128, 1], F32, tag="ones_col")
            nc.vector.memset(ones_col[:], 1.0)
            ones_row = cpool.tile([1, 128], F32, tag="ones_row")
            nc.vector.memset(ones_row[:], 1.0)

            # ---- persistent activations ----
            ktr = apool.tile([128, s], BF16, tag="ktr")
            qtr = {}   # (h, qb) -> tile, created lazily in projection loop
            v_sb = {}  # st -> tile
            otr = {}   # (h, qb) -> tile

            def rope(dsl, src_psum, sc_i):
                """dsl ([128, sc] slice) = rope(src) with de-interleaved halves.

                src rows 0:64 = even dims (a), 64:128 = odd dims (b).
                re = a*c - b*s -> rows 0:64 ; ro = a*s + b*c -> rows 64:128.
                """
                c = cos_sb[:, sc_i * sc:(sc_i + 1) * sc]
                sn = sin_sb[:, sc_i * sc:(sc_i + 1) * sc]
                t1 = tpool.tile([128, sc], F32, tag="t1", bufs=2)
                t2 = tpool.tile([128, sc], F32, tag="t2", bufs=2)
                # Walrus only requires equal base partitions when BOTH inputs
                # are SBUF; the PSUM operand may be partition-shifted, so t2
                # is built with halves pre-swapped (b*s on top, a*s below).
                nc.vector.tensor_mul(t1[0:64, :], src_psum[0:64, :], c)
                nc.vector.tensor_mul(t1[64:128, :], src_psum[64:128, :], c)
                nc.vector.tensor_mul(t2[0:64, :], src_psum[64:128, :], sn)
                nc.vector.tensor_mul(t2[64:128, :], src_psum[0:64, :], sn)
                nc.vector.tensor_sub(dsl[0:64, :], t1[0:64, :], t2[0:64, :])
                nc.vector.tensor_add(dsl[64:128, :], t1[64:128, :], t2[64:128, :])

            # ---- phase 1: projections ----
            for sc_i in range(nsc):
                xt = []
                for kc in range(kc_n):
                    t = xpool.tile([128, sc], BF16, tag=f"x{kc}", bufs=2, name=f"xt{kc}")
                    nc.sync.dma_start(
                        t[:], xT[kc * 128:(kc + 1) * 128, sc_i * sc:(sc_i + 1) * sc]
                    )
                    xt.append(t)
                qb_i, half = sc_i, 0
                for grp in ([0, 1, 2], [3, 4]):  # 4 = the KV head's K proj
                    psums = {}
                    for hh in grp:
                        psums[hh] = pp.tile([128, sc], F32, tag="pp", bufs=3,
                                            name=f"pj{sc_i}_{hh}")
                    for kc in range(kc_n):
                        for hh in grp:
                            if hh < NH:
                                lhsT = wq_sb[:, kc, hh * DH:(hh + 1) * DH]
                            else:
                                lhsT = wkv_sb[:, kc, 0:DH]
                            nc.tensor.matmul(
                                psums[hh][:], lhsT, xt[kc][:],
                                start=(kc == 0), stop=(kc == kc_n - 1),
                            )
                    for hh in grp:
                        if hh < NH:
                            if (hh, qb_i) not in qtr:
                                qtr[(hh, qb_i)] = apool.tile(
                                    [128, 512], BF16, tag=f"qtr{hh}", bufs=2,
                                    name=f"qtr{hh}_{qb_i}")
                            rope(qtr[(hh, qb_i)][:, half * sc:(half + 1) * sc],
                                 psums[hh], sc_i)
                        else:
                            rope(ktr[:, sc_i * sc:(sc_i + 1) * sc], psums[hh], sc_i)
                for stl in range(sc // 128):
                    st = sc_i * (sc // 128) + stl
                    vp = pp.tile([128, 128], F32, tag="pp", bufs=3, name=f"vp{st}")
                    for kc in range(kc_n):
                        nc.tensor.matmul(
                            vp[:], xt[kc][:, stl * 128:(stl + 1) * 128],
                            wkv_sb[:, kc, DH:2 * DH],
                            start=(kc == 0), stop=(kc == kc_n - 1),
                        )
                    v_sb[st] = apool.tile([128, DH], BF16, tag=f"v{st}", name=f"v{st}")
                    nc.scalar.copy(v_sb[st][:], vp[:])

            # ---- phase 2+3: attention + output projection, per q-block ----
            for qb in range(qb_n):
                for h in range(NH):
                    nkc = 4 * (qb + 1)
                    l_acc = tpool.tile([128, 512], F32, tag="lacc", bufs=2,
                                       name=f"lacc{h}_{qb}")
                    otp = pp.tile([128, 512], F32, tag="ot", bufs=1,
                                  name=f"otp{h}_{qb}")
                    for kc in range(nkc):
                        scp = pp.tile([128, 512], F32, tag="sc", bufs=2,
                                      name=f"scp{h}_{qb}_{kc}")
                        nc.tensor.matmul(
                            scp[:], ktr[:, kc * 128:(kc + 1) * 128],
                            qtr[(h, qb)][:], start=True, stop=True,
                        )
                        pt = tpool.tile([128, 512], BF16, tag="pt", bufs=3,
                                        name=f"pt{h}_{qb}_{kc}")
                        nc.scalar.activation(
                            pt[:], scp[:], mybir.ActivationFunctionType.Exp,
                            scale=float(INV_SQRT_DH),
                        )
                        if kc >= 4 * qb:  # diagonal-straddling: zero disallowed
                            stt = 384 - (kc - 4 * qb) * 128
                            nc.gpsimd.tensor_mul(pt[:], pt[:], mask_sb[:, stt:stt + 512])
                        if kc == 0:
                            nc.vector.tensor_copy(l_acc[:], pt[:])
                        else:
                            nc.vector.tensor_add(l_acc[:], l_acc[:], pt[:])
                        nc.tensor.matmul(
                            otp[:], v_sb[kc][:], pt[:],
                            start=(kc == 0), stop=(kc == nkc - 1),
                        )
                    lp = pp.tile([1, 512], F32, tag="lr", bufs=1, name=f"lp{h}_{qb}")
                    nc.tensor.matmul(lp[:], ones_col[:], l_acc[:], start=True, stop=True)
                    rl = tpool.tile([1, 512], F32, tag="rl", bufs=2, name=f"rl{h}_{qb}")
                    nc.vector.reciprocal(rl[:], lp[:])
                    rlb = pp.tile([128, 512], F32, tag="lr", bufs=1, name=f"rlb{h}_{qb}")
                    nc.tensor.matmul(rlb[:], ones_row[:], rl[:], start=True, stop=True)
                    rlb_sb = tpool.tile([128, 512], F32, tag="rlbs", bufs=2,
                                        name=f"rlbs{h}_{qb}")
                    nc.scalar.copy(rlb_sb[:], rlb[:])
                    otr[(h, qb)] = apool.tile([128, 512], F32, tag=f"otr{h}", bufs=2,
                                              name=f"otr{h}_{qb}")
                    nc.vector.tensor_mul(otr[(h, qb)][:], otp[:], rlb_sb[:])

                for stl in range(4):
                    st = 4 * qb + stl
                    for dm in range(d // 512):
                        wop = pp.tile([128, 512], F32, tag="wp", bufs=1,
                                      name=f"wop{st}_{dm}")
                        for h in range(NH):
                            nc.tensor.matmul(
                                wop[:],
                                otr[(h, qb)][:, stl * 128:(stl + 1) * 128],
                                wo_sb[:, h, dm * 512:(dm + 1) * 512],
                                start=(h == 0), stop=(h == NH - 1),
                            )
                        osb = tpool.tile([128, 512], F32, tag="osb", bufs=3,
                                         name=f"osb{st}_{dm}")
                        nc.vector.tensor_copy(osb[:], wop[:])
                        nc.sync.dma_start(
                            out_p[st * 128:(st + 1) * 128, dm * 512:(dm + 1) * 512],
                            osb[:],
                        )

    nc.compile()
    return nc


_PROGRAM = None


def _get_program():
    global _PROGRAM
    if _PROGRAM is None:
        _PROGRAM = build_program()
    return _PROGRAM


_DEINT = np.concatenate([np.arange(0, DH, 2), np.arange(1, DH, 2)])


def make_in_maps(x, rope_cos, rope_sin, Wq, Wk, Wv, Wo, s=S):
    cosT = np.ascontiguousarray(rope_cos[:s].T.astype(np.float32))
    sinT = np.ascontiguousarray(rope_sin[:s].T.astype(np.float32))
    kp = np.arange(128)[:, None]
    cc = np.arange(896)[None, :]
    maskb = (cc >= kp + 384).astype(ml_dtypes.bfloat16)
    in_maps = []
    for c in range(N_CORES):
        b, g = divmod(c, 4)
        xTc = np.ascontiguousarray(x[b].T.astype(ml_dtypes.bfloat16))
        wq_cols = [
            Wq[:, (g * NH + j) * DH:(g * NH + j + 1) * DH][:, _DEINT]
            for j in range(NH)
        ]
        wq_c = np.ascontiguousarray(np.concatenate(wq_cols, axis=1).astype(ml_dtypes.bfloat16))
        wk_c = Wk[:, g * DH:(g + 1) * DH][:, _DEINT]
        wv_c = Wv[:, g * DH:(g + 1) * DH]
        wkv_c = np.ascontiguousarray(
            np.concatenate([wk_c, wv_c], axis=1).astype(ml_dtypes.bfloat16))
        wo_c = np.ascontiguousarray(
            Wo[g * NH * DH:(g + 1) * NH * DH, :].astype(np.float32))
        in_maps.append({
            "xT": xTc, "wq": wq_c, "wkv": wkv_c, "wo": wo_c,
            "cosT": cosT, "sinT": sinT, "maskb": maskb,
        })
    return in_maps


def kernel(x, rope_cos, rope_sin, Wq, Wk, Wv, Wo):
    nc = _get_program()
    in_maps = make_in_maps(x, rope_cos, rope_sin, Wq, Wk, Wv, Wo)
    res = run_bass_kernel_spmd(nc, in_maps, list(range(N_CORES)))
    out = np.zeros((B, S, D), dtype=np.float32)
    for c in range(N_CORES):
        b, g = divmod(c, 4)
        out[b] += res.results[c]["out_p"]
    return out



# revision 6
# speedup vs baseline: 2.2039x; 2.2039x over previous
"""GQA attention kernel for Trainium2, sharded over 8 NeuronCores.

Sharding: core c = b*4 + g handles batch b and GQA group g (4 query heads
+ 1 KV head). Wq/Wk/Wv column-sharded per group, Wo row-sharded; the host
sums the 4 per-group partial outputs per batch.

Device layout:
  - x is passed transposed (xT [D, S]) so Q^T/K^T project directly into
    [head_dim, S] layout (head_dim on partitions) and V projects into
    natural [S, head_dim] layout.
  - Q/K head dims are de-interleaved host-side (even dims then odd dims)
    by permuting Wq/Wk columns; scores are invariant to a shared
    permutation of Q/K dims.  RoPE is then 4 DVE ops per [128,512] chunk
    using pre-duplicated cos ([c;c]) and pre-signed sin ([-s;+s]).
  - Attention computes scoresT [key, query]; softmax exp output is
    directly the lhs^T operand for the P@V matmul.  Scores for two
    adjacent key chunks share one [128,2,512] PSUM tile so exp runs at
    free-dim 1024.  exp carries bias -ln(64) so fp16 probs can't
    overflow; the 1/64 cancels in the softmax normalization.
  - Causal banding at 256-query granularity: diagonal chunk pairs only
    compute the allowed query range; triangular 128-col masks finish the
    job (gpsimd + DVE).
  - Denominator: fp16 chunk accumulation (DVE 2x), ones-matmul partition
    reduce, reciprocal_approx_fast, f32r ones-broadcast back to 128
    partitions.
  - All matmuls are 16-bit (bf16/fp16) except the two tiny f32r
    broadcast/reduce helpers; Wo and otr are bf16.
  - Projection and output-projection matmuls are emitted interleaved
    into the attention loop ("fillers") so the PE FIFO never idles
    behind the ACT-bound exp chain and the HAM clock stays warm.
"""

import sys

if "/opt/trn_rl_repo" not in sys.path:
    sys.path.insert(0, "/opt/trn_rl_repo")

import numpy as np
import ml_dtypes

import concourse.bass as bass
import concourse.bacc as bacc
import concourse.tile as tile
from concourse import mybir
from concourse.bass_utils import run_bass_kernel_spmd

B = 2
S = 2048
D = 2048
N_HEADS = 16
N_KV = 4
DH = 128
NH = 4  # query heads per core
N_CORES = 8

INV_SQRT_DH = 1.0 / np.sqrt(DH)
LN64 = float(np.log(64.0))
F32 = mybir.dt.float32
F32R = mybir.dt.float32r
BF16 = mybir.dt.bfloat16
FP16 = mybir.dt.float16


def build_program(s=S, d=D):
    """Per-core program: 4 query heads + 1 KV head of causal GQA."""
    kc_n = d // 128       # contraction chunks
    qb_n = s // 512       # q-blocks / s-chunks

    nc = bacc.Bacc("TRN2", target_bir_lowering=False, debug=False,
                   num_devices=N_CORES)
    xT = nc.declare_dram_parameter("xT", [d, s], BF16, isOutput=False)
    wq = nc.declare_dram_parameter("wq", [d, NH * DH], BF16, isOutput=False)
    wkv = nc.declare_dram_parameter("wkv", [d, 2 * DH], BF16, isOutput=False)
    wo = nc.declare_dram_parameter("wo", [NH * DH, d], BF16, isOutput=False)
    cosD = nc.declare_dram_parameter("cosD", [128, s], BF16, isOutput=False)
    sinS = nc.declare_dram_parameter("sinS", [128, s], BF16, isOutput=False)
    maskA = nc.declare_dram_parameter("maskA", [128, 128], FP16, isOutput=False)
    maskB = nc.declare_dram_parameter("maskB", [128, 256], FP16, isOutput=False)
    out_p = nc.declare_dram_parameter("out_p", [s, d], F32, isOutput=True)

    with tile.TileContext(nc) as tc:
        with (
            tc.tile_pool(name="const", bufs=1) as cpool,
            tc.tile_pool(name="xp", bufs=1) as xpool,
            tc.tile_pool(name="act", bufs=1) as apool,
            tc.tile_pool(name="tmp", bufs=1) as tpool,
            tc.tile_pool(name="psum", bufs=1, space="PSUM") as pp,
        ):
            # ---- constants ----
            wq_sb = cpool.tile([128, kc_n, NH * DH], BF16, tag="wq")
            nc.sync.dma_start(wq_sb[:], wq.rearrange("(n p) m -> p n m", p=128))
            wkv_sb = cpool.tile([128, kc_n, 2 * DH], BF16, tag="wkv")
            nc.sync.dma_start(wkv_sb[:], wkv.rearrange("(n p) m -> p n m", p=128))
            wo_sb = cpool.tile([128, NH, d], BF16, tag="wo")
            nc.sync.dma_start(wo_sb[:], wo.rearrange("(n p) m -> p n m", p=128))
            cos_sb = cpool.tile([128, s], BF16, tag="cos")
            nc.sync.dma_start(cos_sb[:], cosD[:])
            sin_sb = cpool.tile([128, s], BF16, tag="sin")
            nc.sync.dma_start(sin_sb[:], sinS[:])
            mA = cpool.tile([128, 128], FP16, tag="mA")
            nc.sync.dma_start(mA[:], maskA[:])
            mB = cpool.tile([128, 256], FP16, tag="mB")
            nc.sync.dma_start(mB[:], maskB[:])
            ones_col = cpool.tile([128, 1], FP16, tag="ones_col")
            nc.vector.memset(ones_col[:], 1.0)
            ones_row = cpool.tile([1, 128], FP16, tag="ones_row")
            nc.vector.memset(ones_row[:], 1.0)
            nln64 = cpool.tile([128, 1], F32, tag="nln64")
            nc.vector.memset(nln64[:], -LN64)

            # ---- persistent activations ----
            ktr = apool.tile([128, s], BF16, tag="ktr")
            qtr = {}   # (h, qb) -> tile
            v_sb = {}  # st -> tile
            otr = {}   # (h, qb) -> tile

            def rope(dst, src_psum, sc_i):
                """dst [128,512] bf16 = rope(src) with de-interleaved halves.

                src rows 0:64 = even dims (a), 64:128 = odd dims (b).
                re = a*c - b*s -> rows 0:64 ; ro = a*s + b*c -> rows 64:128.
                cos_sb = [c; c], sin_sb = [-s; +s] so this is
                  t1 = src * cos_sb      (one [128,512] op)
                  t2 = swap(src) * sin_sb (two [64,512] ops, psum shifted)
                  dst = t1 + t2          (one [128,512] bf16 2x op)
                """
                c = cos_sb[:, sc_i * 512:(sc_i + 1) * 512]
                sg = sin_sb[:, sc_i * 512:(sc_i + 1) * 512]
                t1 = tpool.tile([128, 512], BF16, tag="t1", bufs=2)
                t2 = tpool.tile([128, 512], BF16, tag="t2", bufs=2)
                nc.vector.tensor_mul(t1[:], src_psum[:], c)
                nc.vector.tensor_mul(t2[0:64, :], src_psum[64:128, :], sg[0:64, :])
                nc.vector.tensor_mul(t2[64:128, :], src_psum[0:64, :], sg[64:128, :])
                nc.vector.tensor_add(dst[:], t1[:], t2[:])

            def gen_proj(sc_i):
                """Projection phase for s-chunk sc_i; yields after each PE op."""
                xt = xpool.tile([128, kc_n, 512], BF16, tag="xt", bufs=2,
                                name=f"xt{sc_i}")
                xv = xT.rearrange("(n p) m -> p n m", p=128)
                for j4 in range(0, kc_n, 4):
                    nc.sync.dma_start(
                        xt[:, j4:j4 + 4, :],
                        xv[:, j4:j4 + 4, sc_i * 512:(sc_i + 1) * 512])
                # 5 single-head groups: q0..q3 then k (pp rotation overlaps
                # head i+1's matmuls with rope(head i) on DVE)
                for hh in range(NH + 1):
                    ps = pp.tile([128, 512], F32, tag="pp", bufs=2,
                                 name=f"pj{sc_i}_{hh}")
                    for kc in range(kc_n):
                        if hh < NH:
                            lhsT = wq_sb[:, kc, hh * DH:(hh + 1) * DH]
                        else:
                            lhsT = wkv_sb[:, kc, 0:DH]
                        nc.tensor.matmul(ps[:], lhsT, xt[:, kc, :],
                                         start=(kc == 0), stop=(kc == kc_n - 1))
                        yield
                    if hh < NH:
                        qtr[(hh, sc_i)] = apool.tile(
                            [128, 512], BF16, tag=f"qtr{hh}", bufs=2,
                            name=f"qtr{hh}_{sc_i}")
                        rope(qtr[(hh, sc_i)][:], ps, sc_i)
                    else:
                        rope(ktr[:, sc_i * 512:(sc_i + 1) * 512], ps, sc_i)
                for stl in range(4):
                    st = sc_i * 4 + stl
                    vp = pp.tile([128, 128], F32, tag="pp", bufs=2,
                                 name=f"vp{st}")
                    for kc in range(kc_n):
                        nc.tensor.matmul(
                            vp[:], xt[:, kc, stl * 128:(stl + 1) * 128],
                            wkv_sb[:, kc, DH:2 * DH],
                            start=(kc == 0), stop=(kc == kc_n - 1))
                        yield
                    v_sb[st] = apool.tile([128, DH], FP16, tag=f"v{st}",
                                          name=f"v{st}")
                    nc.scalar.copy(v_sb[st][:], vp[:])

            def gen_outproj(qb):
                """Output projection for q-block qb; yields after each PE op."""
                for stl in range(4):
                    st = 4 * qb + stl
                    for dm in range(4):
                        wop = pp.tile([128, 512], F32, tag="pp", bufs=2,
                                      name=f"wop{st}_{dm}")
                        for h in range(NH):
                            nc.tensor.matmul(
                                wop[:],
                                otr[(h, qb)][:, stl * 128:(stl + 1) * 128],
                                wo_sb[:, h, dm * 512:(dm + 1) * 512],
                                start=(h == 0), stop=(h == NH - 1))
                            yield
                        osb = tpool.tile([128, 512], F32, tag="osb", bufs=4,
                                         name=f"osb{st}_{dm}")
                        if (st + dm) % 2 == 0:
                            nc.vector.tensor_copy(osb[:], wop[:])
                        else:
                            nc.scalar.copy(osb[:], wop[:])
                        nc.sync.dma_start(
                            out_p[st * 128:(st + 1) * 128,
                                  dm * 512:(dm + 1) * 512], osb[:])

            fillers = []

            def pull(n):
                for _ in range(n):
                    while fillers:
                        try:
                            next(fillers[0])
                            break
                        except StopIteration:
                            fillers.pop(0)
                    else:
                        return

            def drain():
                while fillers:
                    try:
                        next(fillers[0])
                    except StopIteration:
                        fillers.pop(0)

            def attn(qb):
                """Attention for q-block qb (4 heads), pulling PE fillers."""
                npair = 2 * (qb + 1)
                for h in range(NH):
                    q_t = qtr[(h, qb)]
                    l_acc = tpool.tile([128, 512], FP16, tag="lacc", bufs=2,
                                       name=f"lacc{h}_{qb}")
                    otp = pp.tile([128, 512], F32, tag="ot", bufs=2,
                                  name=f"otp{h}_{qb}")
                    pts = []
                    # scores + exp + mask + denominator accumulate
                    for j in range(npair):
                        jd = j - 2 * qb  # >=0 on diagonal pairs
                        qo = 256 * jd if jd > 0 else 0
                        scp = pp.tile([128, 2, 512], F32, tag="sc", bufs=2,
                                      name=f"scp{h}_{qb}_{j}")
                        nc.tensor.matmul(scp[:, 0, qo:],
                                         ktr[:, (2 * j) * 128:(2 * j + 1) * 128],
                                         q_t[:, qo:], start=True, stop=True)
                        pull(2)
                        nc.tensor.matmul(scp[:, 1, qo:],
                                         ktr[:, (2 * j + 1) * 128:(2 * j + 2) * 128],
                                         q_t[:, qo:], start=True, stop=True)
                        pull(2)
                        pt = tpool.tile([128, 2, 512], FP16, tag="pt", bufs=9,
                                        name=f"pt{h}_{qb}_{j}")
                        pts.append((pt, qo))
                        nc.scalar.activation(
                            pt[:, :, qo:], scp[:, :, qo:],
                            mybir.ActivationFunctionType.Exp,
                            scale=float(INV_SQRT_DH), bias=nln64[:])
                        if jd >= 0:
                            # chunk 2j: triangle in cols [qo, qo+128)
                            nc.gpsimd.tensor_mul(pt[:, 0, qo:qo + 128],
                                                 pt[:, 0, qo:qo + 128], mA[:])
                            # chunk 2j+1: first 128 cols dead + triangle
                            nc.vector.tensor_mul(pt[:, 1, qo:qo + 256],
                                                 pt[:, 1, qo:qo + 256], mB[:])
                        if j == 0:
                            nc.vector.tensor_copy(l_acc[:], pt[:, 0, :])
                        else:
                            nc.vector.tensor_add(l_acc[:, qo:], l_acc[:, qo:],
                                                 pt[:, 0, qo:])
                        nc.vector.tensor_add(l_acc[:, qo:], l_acc[:, qo:],
                                             pt[:, 1, qo:])
                    # P @ V (dense on PE)
                    for j in range(npair):
                        pt, qo = pts[j]
                        nc.tensor.matmul(otp[:, qo:], v_sb[2 * j][:],
                                         pt[:, 0, qo:], start=(j == 0),
                                         stop=False)
                        nc.tensor.matmul(otp[:, qo:], v_sb[2 * j + 1][:],
                                         pt[:, 1, qo:], start=False,
                                         stop=(j == npair - 1))
                    # denominator -> reciprocal -> broadcast -> normalize
                    lpt = pp.tile([128, 2, 512], F32, tag="sc", bufs=2,
                                  name=f"lp{h}_{qb}")
                    nc.tensor.matmul(lpt[0:1, 0, :], ones_col[:], l_acc[:],
                                     start=True, stop=True)
                    pull(2)
                    rl = tpool.tile([1, 512], F32, tag="rl", bufs=2,
                                    name=f"rl{h}_{qb}")
                    nc.vector.reciprocal_approx_fast(rl[:], lpt[0:1, 0, :])
                    rl16 = tpool.tile([1, 512], FP16, tag="rl16", bufs=2,
                                      name=f"rl16{h}_{qb}")
                    nc.scalar.copy(rl16[:], rl[:])
                    rlbt = pp.tile([128, 2, 512], F32, tag="sc", bufs=2,
                                   name=f"rlb{h}_{qb}")
                    nc.tensor.matmul(rlbt[:, 0, :], ones_row[:], rl16[:],
                                     start=True, stop=True)
                    pull(2)
                    rlb_sb = tpool.tile([128, 512], F32, tag="rlbs", bufs=2,
                                        name=f"rlbs{h}_{qb}")
                    nc.scalar.copy(rlb_sb[:], rlbt[:, 0, :])
                    otr[(h, qb)] = apool.tile([128, 512], BF16, tag=f"otr{h}",
                                              bufs=2, name=f"otr{h}_{qb}")
                    nc.vector.tensor_mul(otr[(h, qb)][:], otp[:], rlb_sb[:])

            # ---- main schedule ----
            for _ in gen_proj(0):
                pass
            for qb in range(qb_n):
                if qb > 0:
                    fillers.append(gen_outproj(qb - 1))
                if qb + 1 < qb_n:
                    fillers.append(gen_proj(qb + 1))
                attn(qb)
                drain()
            for _ in gen_outproj(qb_n - 1):
                pass

    nc.compile()
    return nc


_PROGRAM = None


def _get_program():
    global _PROGRAM
    if _PROGRAM is None:
        _PROGRAM = build_program()
    return _PROGRAM


_DEINT = np.concatenate([np.arange(0, DH, 2), np.arange(1, DH, 2)])


def make_in_maps(x, rope_cos, rope_sin, Wq, Wk, Wv, Wo, s=S):
    cosT = rope_cos[:s].T.astype(np.float32)   # [64, s]
    sinT = rope_sin[:s].T.astype(np.float32)
    cosD = np.concatenate([cosT, cosT], axis=0).astype(ml_dtypes.bfloat16)
    sinS = np.concatenate([-sinT, sinT], axis=0).astype(ml_dtypes.bfloat16)
    p = np.arange(128)[:, None]
    maskA = (np.arange(128)[None, :] >= p).astype(np.float16)
    maskB = (np.arange(256)[None, :] >= p + 128).astype(np.float16)
    in_maps = []
    for c in range(N_CORES):
        b, g = divmod(c, 4)
        xTc = np.ascontiguousarray(x[b].T.astype(ml_dtypes.bfloat16))
        wq_cols = [
            Wq[:, (g * NH + j) * DH:(g * NH + j + 1) * DH][:, _DEINT]
            for j in range(NH)
        ]
        wq_c = np.ascontiguousarray(
            np.concatenate(wq_cols, axis=1).astype(ml_dtypes.bfloat16))
        wk_c = Wk[:, g * DH:(g + 1) * DH][:, _DEINT]
        wv_c = Wv[:, g * DH:(g + 1) * DH]
        wkv_c = np.ascontiguousarray(
            np.concatenate([wk_c, wv_c], axis=1).astype(ml_dtypes.bfloat16))
        wo_c = np.ascontiguousarray(
            Wo[g * NH * DH:(g + 1) * NH * DH, :].astype(ml_dtypes.bfloat16))
        in_maps.append({
            "xT": xTc, "wq": wq_c, "wkv": wkv_c, "wo": wo_c,
            "cosD": np.ascontiguousarray(cosD),
            "sinS": np.ascontiguousarray(sinS),
            "maskA": maskA, "maskB": maskB,
        })
    return in_maps


def kernel(x, rope_cos, rope_sin, Wq, Wk, Wv, Wo):
    nc = _get_program()
    in_maps = make_in_maps(x, rope_cos, rope_sin, Wq, Wk, Wv, Wo)
    res = run_bass_kernel_spmd(nc, in_maps, list(range(N_CORES)))
    out = np.zeros((B, S, D), dtype=np.float32)
    for c in range(N_CORES):
        b, g = divmod(c, 4)
        out[b] += res.results[c]["out_p"]
    return out
